# revision 27
# baseline (speedup 1.0000x reference)
"""Causal multi-head self-attention with RoPE on 8 NeuronCores.

Sharding: batch (4) x head-halves (2) -> 8 cores. Core c handles batch c//2,
heads [8*(c%2), 8*(c%2)+8). Pipeline: for each 512-row sequence chunk n
(ascending), QKV+RoPE for chunk n, then attention rows n against all j<=n,
then an AllGather of the bf16 o^T chunk across the core pair, then the
output projection column-sharded over Wo (each core produces y[:, 512 cols]).
Host concatenates the two column halves. No AllReduce.
"""

import numpy as np
import ml_dtypes

import concourse.bacc as bacc
import concourse.bass as bass
import concourse.mybir as mybir
from concourse.tile import TileContext
from concourse.bass_utils import run_bass_kernel_spmd

B, S, D, H = 4, 2048, 1024, 16
HL = 8          # heads per core
DK = 64         # head dim
NCORES = 8
DT = D // 128   # 8 d-tiles (contraction tiles)
OT = HL * DK // 128   # 4 o-tiles for Q^T/K^T ([128, S] each, 2 heads per tile)
ST = S // 128   # 16 s-tiles
NCH = S // 512  # 4 sequence chunks of 512
VW = 66         # V columns per head: 64 data + ones + zero pad (alignment)

BF16 = mybir.dt.bfloat16
F32 = mybir.dt.float32
NEG = -1.0e9

_compiled = {}


def _build_nc():
    nc = bacc.Bacc("TRN2", target_bir_lowering=False, debug=False,
                   num_devices=NCORES)

    xT = nc.dram_tensor("xT", [D, S], BF16, kind="ExternalInput")
    wqT = nc.dram_tensor("wqT", [D, HL * DK], BF16, kind="ExternalInput")
    wkT = nc.dram_tensor("wkT", [D, HL * DK], BF16, kind="ExternalInput")
    wvT = nc.dram_tensor("wvT", [D, HL * DK], BF16, kind="ExternalInput")
    woT2 = nc.dram_tensor("woT2", [D, 512], BF16, kind="ExternalInput")
    cosT = nc.dram_tensor("cosT", [128, S], BF16, kind="ExternalInput")
    sinT = nc.dram_tensor("sinT", [128, S], BF16, kind="ExternalInput")
    swapT = nc.dram_tensor("swapT", [128, 128], BF16, kind="ExternalInput")
    maskT = nc.dram_tensor("maskT", [128, 128], F32, kind="ExternalInput")
    selT = nc.dram_tensor("selT", [2, 128], F32, kind="ExternalInput")
    y = nc.dram_tensor("y", [S, 512], BF16, kind="ExternalOutput")

    groups = [[0, 1], [2, 3], [4, 5], [6, 7]]

    with TileContext(nc) as tc:
        with (
            tc.tile_pool(name="big", bufs=1) as big,
            tc.tile_pool(name="work", bufs=2) as work,
            tc.tile_pool(name="ptile", bufs=24) as ptile,
            tc.tile_pool(name="norm", bufs=1) as normp,
            tc.tile_pool(name="ps_b", bufs=2, space="PSUM") as ps_b,
            tc.tile_pool(name="ps_o", bufs=2, space="PSUM") as ps_o,
            tc.tile_pool(name="dram", bufs=1, space="DRAM") as dram,
        ):
            # ---- persistent SBUF tiles ----
            w_sb = {"q": [], "k": [], "v": []}
            for name, drt in (("q", wqT), ("k", wkT), ("v", wvT)):
                for k in range(DT):
                    t = big.tile([128, HL * DK], BF16, tag=f"w{name}{k}",
                                 name=f"w{name}{k}")
                    w_sb[name].append(t)
            wo_sb = [big.tile([128, 512], BF16, tag=f"wo{k}", name=f"wo{k}")
                     for k in range(DT)]
            xT_sb = [big.tile([128, S], BF16, tag=f"xT{k}", name=f"xT{k}")
                     for k in range(DT)]
            cos_sb = big.tile([128, S], BF16, tag="cos")
            sin_sb = big.tile([128, S], BF16, tag="sin")
            swap_sb = big.tile([128, 128], BF16, tag="swap")
            mask_sb = big.tile([128, 128], F32, tag="mask")
            qrot_sb = [big.tile([128, S], BF16, tag=f"qr{t}", name=f"qr{t}")
                       for t in range(OT)]
            krot_sb = [big.tile([128, S], BF16, tag=f"kr{t}", name=f"kr{t}")
                       for t in range(OT)]
            vaug_sb = [big.tile([128, HL * VW], BF16, tag=f"va{st}",
                                name=f"va{st}") for st in range(ST)]

            # ---- upfront work: memsets first (no deps), then DMAs in
            # consumption order, spread across the three DMA-capable queues.
            for st in range(ST):
                v3 = vaug_sb[st][:].rearrange("p (h d) -> p h d", d=VW)
                nc.gpsimd.memset(v3[:, :, DK:DK + 1], 1.0)
                nc.gpsimd.memset(v3[:, :, DK + 1:VW], 0.0)
            sel_sb = big.tile([2, 128], F32, tag="sel")
            # interleave loads in the order the first Q/K chains consume
            # them: scalar wq0-3+wk0-3+tables; sync xT-even+wq4-7; gpsimd
            # xT-odd+wk4-7
            for k in range(4):
                nc.scalar.dma_start(out=w_sb["q"][k][:],
                                    in_=wqT[128 * k:128 * (k + 1), :])
            def ldx(k, eng):
                eng.dma_start(out=xT_sb[k][:, 0:512],
                              in_=xT[128 * k:128 * (k + 1), 0:512])
            def ldw(name, k, eng):
                drt = {"q": wqT, "k": wkT, "v": wvT}[name]
                eng.dma_start(out=w_sb[name][k][:],
                              in_=drt[128 * k:128 * (k + 1), :])
            ldx(0, nc.sync); ldw("q", 4, nc.sync); ldx(2, nc.sync)
            ldw("q", 5, nc.sync); ldx(4, nc.sync); ldw("q", 6, nc.sync)
            ldx(6, nc.sync); ldw("q", 7, nc.sync)
            ldx(1, nc.gpsimd); ldx(3, nc.gpsimd); ldw("k", 4, nc.gpsimd)
            ldw("k", 5, nc.gpsimd); ldx(5, nc.gpsimd); ldw("k", 6, nc.gpsimd)
            ldx(7, nc.gpsimd); ldw("k", 7, nc.gpsimd)
            for k in range(4):
                nc.scalar.dma_start(out=w_sb["k"][k][:],
                                    in_=wkT[128 * k:128 * (k + 1), :])
            nc.scalar.dma_start(out=sin_sb[:], in_=sinT[:])
            nc.scalar.dma_start(out=swap_sb[:], in_=swapT[:])
            nc.scalar.dma_start(out=cos_sb[:], in_=cosT[:])
            nc.scalar.dma_start(out=sel_sb[:], in_=selT[:])
            for k in range(DT):
                nc.sync.dma_start(out=w_sb["v"][k][:],
                                  in_=wvT[128 * k:128 * (k + 1), :])
            nc.sync.dma_start(out=mask_sb[:], in_=maskT[:])
            for k in range(DT):
                nc.gpsimd.dma_start(out=wo_sb[k][:],
                                    in_=woT2[128 * k:128 * (k + 1), :])

            # per-(chunk, head-pair) DRAM buffers for the o^T exchange
            ocp = [[dram.tile([128, 512], BF16, tag=f"ocp{n}_{tp}",
                              name=f"ocp{n}_{tp}")
                    for tp in range(OT)] for n in range(NCH)]
            ocg = [[dram.tile([256, 512], BF16, tag=f"ocg{n}_{tp}",
                              name=f"ocg{n}_{tp}")
                    for tp in range(OT)] for n in range(NCH)]

            def qkv_chunk(n):
                sl = slice(512 * n, 512 * (n + 1))
                # prefetch next chunk's xT columns
                if n + 1 < NCH:
                    sl2 = slice(512 * (n + 1), 512 * (n + 2))
                    for k in range(DT):
                        eng = nc.sync if k % 2 == 0 else nc.gpsimd
                        eng.dma_start(out=xT_sb[k][:, sl2],
                                      in_=xT[128 * k:128 * (k + 1), sl2])
                for wname, rot in (("q", qrot_sb), ("k", krot_sb)):
                    pss = []
                    for tpair in ((0, 1), (2, 3)):
                        ps = ps_b.tile([128, 1024], F32, tag="psb", name="ps")
                        pss.append(ps)
                        for k in range(DT):
                            for i, t in enumerate(tpair):
                                nc.tensor.matmul(
                                    ps[:, 512 * i:512 * (i + 1)],
                                    lhsT=w_sb[wname][k][:, 128 * t:128 * (t + 1)],
                                    rhs=xT_sb[k][:, sl],
                                    start=(k == 0), stop=(k == DT - 1),
                                )
                    for pi, tpair in enumerate(((0, 1), (2, 3))):
                        ps = pss[pi]
                        tmps, t1s = [], []
                        for i, t in enumerate(tpair):
                            psl = ps[:, 512 * i:512 * (i + 1)]
                            tmp = work.tile([128, 512], BF16, tag="tmp", name="tmp")
                            nc.vector.tensor_mul(tmp[:], psl, sin_sb[:, sl])
                            t1 = work.tile([128, 512], BF16, tag="t1", name="t1")
                            nc.vector.tensor_mul(t1[:], psl, cos_sb[:, sl])
                            tmps.append(tmp)
                            t1s.append(t1)
                        tag2 = "psoe" if pi == 0 else "psoo"
                        for i, t in enumerate(tpair):
                            ps2 = ps_o.tile([128, 512], F32, tag=tag2,
                                            name="ps2")
                            nc.tensor.matmul(ps2[:], lhsT=swap_sb[:],
                                             rhs=tmps[i][:],
                                             start=True, stop=True)
                            nc.vector.tensor_add(rot[t][:, sl], t1s[i][:],
                                                 ps2[:])
                # V for the 4 s-tiles of this chunk
                for stp in ((4 * n, 4 * n + 1), (4 * n + 2, 4 * n + 3)):
                    ps = ps_b.tile([128, 1024], F32, tag="psb", name="psv")
                    for k in range(DT):
                        for i, st in enumerate(stp):
                            nc.tensor.matmul(
                                ps[:, 512 * i:512 * (i + 1)],
                                lhsT=xT_sb[k][:, 128 * st:128 * (st + 1)],
                                rhs=w_sb["v"][k][:],
                                start=(k == 0), stop=(k == DT - 1),
                            )
                    for i, st in enumerate(stp):
                        dst = vaug_sb[st][:].rearrange(
                            "p (h d) -> p h d", d=VW)[:, :, 0:DK]
                        src = ps[:, 512 * i:512 * (i + 1)].rearrange(
                            "p (h d) -> p h d", d=DK)
                        nc.vector.tensor_copy(dst, src)

            def load_og(pm):
                og = {}
                for tp2 in range(OT):
                    for half in (0, 1):
                        t = work.tile([128, 512], BF16, tag=f"og{tp2}_{half}",
                                      name=f"og{tp2}_{half}", bufs=1)
                        nc.gpsimd.dma_start(
                            out=t[:],
                            in_=ocg[pm][tp2][128 * half:128 * (half + 1), :])
                        og[(tp2, half)] = t
                return og

            def proj_gen(pm, og, pos_major=False):
                # generator of tensor-work thunks for the projection of
                # chunk pm. pos_major: both chains advance together so a
                # late gather only blocks the final positions (used for
                # the very last projection). Otherwise chain-major so the
                # first chain's PSUM frees early for the next QKV chunk.
                korder = [(tp2, half) for tp2 in range(OT) for half in (0, 1)]

                def fin1(yp, rp):
                    for i, r in enumerate(rp):
                        ych = work.tile([128, 512], BF16, tag="ych",
                                        name="ych")
                        if i == 0:
                            nc.vector.tensor_copy(
                                ych[:], yp[:, 512 * i:512 * (i + 1)])
                        else:
                            nc.scalar.copy(
                                ych[:], yp[:, 512 * i:512 * (i + 1)])
                        nc.sync.dma_start(
                            out=y[512 * pm + 128 * r:512 * pm + 128 * (r + 1), :],
                            in_=ych[:])

                if pos_major:
                    yps = [ps_b.tile([128, 1024], F32, tag="psb", name="yp")
                           for _ in range(2)]
                    for pos, (tp2, half) in enumerate(korder):
                        kk = tp2 + 4 * half
                        for ci, rp in enumerate(((0, 1), (2, 3))):
                            for i, r in enumerate(rp):
                                yield lambda yp=yps[ci], i=i, r=r, tp2=tp2, \
                                    half=half, kk=kk, pos=pos: nc.tensor.matmul(
                                        yp[:, 512 * i:512 * (i + 1)],
                                        lhsT=og[(tp2, half)][:, 128 * r:128 * (r + 1)],
                                        rhs=wo_sb[kk][:],
                                        start=(pos == 0), stop=(pos == 7))
                    yield lambda: (fin1(yps[0], (0, 1)), fin1(yps[1], (2, 3)))
                else:
                    for ci, rp in enumerate(((0, 1), (2, 3))):
                        yp = ps_b.tile([128, 1024], F32, tag="psb", name="yp")
                        for pos, (tp2, half) in enumerate(korder):
                            kk = tp2 + 4 * half
                            for i, r in enumerate(rp):
                                yield lambda yp=yp, i=i, r=r, tp2=tp2, \
                                    half=half, kk=kk, pos=pos: nc.tensor.matmul(
                                        yp[:, 512 * i:512 * (i + 1)],
                                        lhsT=og[(tp2, half)][:, 128 * r:128 * (r + 1)],
                                        rhs=wo_sb[kk][:],
                                        start=(pos == 0), stop=(pos == 7))
                        yield lambda yp=yp, rp=rp: fin1(yp, rp)

            def attn_chunk(m, pm=None):
                i0 = 512 * m
                njb = 4 * m + 4
                filler = proj_gen(pm, load_og(pm)) if pm is not None else None

                def emit_score(tp, jb, pTs, av=None):
                    j0 = 128 * jb
                    dlt = max(0, j0 - i0)
                    s_ps = ps_b.tile([128, 1024], F32, tag="psb", name="s_ps")
                    for half, po in ((0, 0), (1, DK)):
                        nc.tensor.matmul(
                            s_ps[:, 512 * half + dlt:512 * (half + 1)],
                            lhsT=krot_sb[tp][po:po + DK, j0:j0 + 128],
                            rhs=qrot_sb[tp][po:po + DK, i0 + dlt:i0 + 512],
                            start=True, stop=True,
                        )
                        if av is not None:
                            av(jb, halves=(half,))
                    if j0 >= i0:
                        s3 = s_ps[:].rearrange("p (b f) -> p b f", b=2)
                        nc.vector.tensor_add(
                            s3[:, :, dlt:dlt + 128],
                            s3[:, :, dlt:dlt + 128],
                            mask_sb[:].rearrange("p (b f) -> p b f", b=1)
                            .broadcast_to([128, 2, 128]))
                    pT = ptile.tile([128, 1024], BF16, tag="pT", name="pT")
                    nc.scalar.activation(
                        pT[:].rearrange("p (b f) -> p b f", b=2)[:, :, dlt:512],
                        s_ps[:].rearrange("p (b f) -> p b f", b=2)[:, :, dlt:512],
                        mybir.ActivationFunctionType.Exp, scale=0.125)
                    pTs.append(pT)

                def make_av(tp, pTs):
                    o_pse = ps_o.tile([VW, 512], F32, tag="psoe", name="o_pse")
                    o_pso = ps_o.tile([VW, 512], F32, tag="psoo", name="o_pso")

                    def th(jb, halves=(0, 1), tp=tp):
                        dlt = max(0, 128 * jb - i0)
                        for half in halves:
                            o_ps = o_pse if half == 0 else o_pso
                            nc.tensor.matmul(
                                o_ps[:, dlt:512],
                                lhsT=vaug_sb[jb][:, VW * (2 * tp + half):
                                                 VW * (2 * tp + half) + VW],
                                rhs=pTs[jb][:, 512 * half + dlt:512 * (half + 1)],
                                start=(jb == 0), stop=(jb == njb - 1),
                            )
                    return th, o_pse, o_pso

                def normalize(tp, o_pse, o_pso, last=False):
                    sums2 = normp.tile([2, 512], F32, tag="sums2", name="sums2")
                    osb = {}
                    for half, o_ps in ((0, o_pse), (1, o_pso)):
                        t = normp.tile([VW - 1, 512], F32, tag=f"osb{half}",
                                       name=f"osb{half}")
                        nc.vector.tensor_copy(t[:], o_ps[0:VW - 1, :])
                        osb[half] = t
                        nc.sync.dma_start(out=sums2[half:half + 1, :],
                                          in_=t[DK:DK + 1, :])
                    rec2 = normp.tile([2, 512], F32, tag="rec2", name="rec2")
                    nc.vector.reciprocal_approx_fast(out=rec2[:], in_=sums2[:])
                    oTn = work.tile([128, 512], BF16, tag=f"oTn{tp}",
                                    name=f"oTn{tp}", bufs=1)
                    if last:
                        # tensor queue is idle here: broadcast 1/sums via a
                        # tiny fp32 matmul (rep[p,i] = sel[h,p]*rec2[h,i])
                        repm = ps_o.tile([128, 512], F32, tag="psoe",
                                         name="repm")
                        nc.tensor.matmul(repm[:], lhsT=sel_sb[:], rhs=rec2[:],
                                         start=True, stop=True)
                        for half in (0, 1):
                            nc.vector.tensor_mul(
                                oTn[64 * half:64 * half + 64, :],
                                osb[half][0:DK, :],
                                repm[64 * half:64 * half + 64, :])
                            nc.gpsimd.dma_start(
                                out=ocp[m][tp][64 * half:64 * (half + 1), :],
                                in_=oTn[64 * half:64 * half + 64, :])
                    else:
                        stage = normp.tile([1, 512], F32, tag="stage",
                                           name="stage")
                        nc.sync.dma_start(out=stage[:], in_=rec2[1:2, :])
                        for half in (0, 1):
                            rep = normp.tile([DK, 512], F32, tag=f"rep{half}",
                                             name=f"rep{half}")
                            src = rec2[0:1, :] if half == 0 else stage[:]
                            nc.gpsimd.partition_broadcast(rep[:], src)
                            nc.vector.tensor_mul(
                                oTn[64 * half:64 * half + 64, :],
                                osb[half][0:DK, :], rep[:])
                            nc.gpsimd.dma_start(
                                out=ocp[m][tp][64 * half:64 * (half + 1), :],
                                in_=oTn[64 * half:64 * half + 64, :])
                    nc.gpsimd.collective_compute(
                        "AllGather", mybir.AluOpType.bypass,
                        replica_groups=groups,
                        ins=[ocp[m][tp][:].opt()],
                        outs=[ocg[m][tp][:].opt()],
                    )

                prev = None  # (tp, pTs)
                for tp in range(OT):
                    pTs = []
                    av = None
                    if prev is not None:
                        ptp, ppTs = prev
                        av, o_e, o_o = make_av(ptp, ppTs)
                    for jb in range(njb):
                        emit_score(tp, jb, pTs, av=av)
                    if av is not None:
                        normalize(ptp, o_e, o_o)
                    prev = (tp, pTs)
                # tail: AV of the last head pair, interleaved with the
                # previous chunk's projection matmuls
                ptp, ppTs = prev
                av, o_e, o_o = make_av(ptp, ppTs)
                for jb in range(njb):
                    av(jb)
                    if filler is not None:
                        for _ in range(4):
                            th = next(filler, None)
                            if th is not None:
                                th()
                normalize(ptp, o_e, o_o, last=(m == NCH - 1))
                if filler is not None:
                    for th in filler:
                        th()

            def proj_chunk(m):
                og = load_og(m)
                for th in proj_gen(m, og, pos_major=True):
                    th()

            qkv_chunk(0)
            attn_chunk(0)
            qkv_chunk(1)
            attn_chunk(1, pm=0)
            qkv_chunk(2)
            attn_chunk(2, pm=1)
            qkv_chunk(3)
            attn_chunk(3, pm=2)
            proj_chunk(3)

    nc.compile()
    return nc


def _prep_inputs(x, Wq, Wk, Wv, Wo, cos_emb, sin_emb, token_positions):
    bf = ml_dtypes.bfloat16
    cos_g = np.asarray(cos_emb)[np.asarray(token_positions)]  # [S, DK]
    sin_g = np.asarray(sin_emb)[np.asarray(token_positions)]
    # [128, S]: partition p -> head-dim p % 64
    cosT = np.ascontiguousarray(np.tile(cos_g.T, (2, 1))).astype(bf)
    sinT = np.ascontiguousarray(np.tile(sin_g.T, (2, 1))).astype(bf)
    # rotate-half-interleaved as a matmul: rh = SWAP @ q (per 128-dim tile)
    swap = np.zeros((128, 128), np.float32)
    for j in range(64):
        swap[2 * j, 2 * j + 1] = -1.0
        swap[2 * j + 1, 2 * j] = 1.0
    swapT = np.ascontiguousarray(swap.T).astype(bf)
    # causal mask for the diagonal 128x128 block in S^T=[j,i] layout
    jj = np.arange(128)[:, None]
    ii = np.arange(128)[None, :]
    maskT = np.where(ii >= jj, 0.0, NEG).astype(np.float32)
    selT = np.zeros((2, 128), np.float32)
    selT[0, 0:64] = 1.0
    selT[1, 64:128] = 1.0

    in_maps = []
    for c in range(NCORES):
        b, hh = c // 2, c % 2
        cols = slice(512 * hh, 512 * (hh + 1))
        in_maps.append({
            "xT": np.ascontiguousarray(np.asarray(x)[b].T).astype(bf),
            "wqT": np.ascontiguousarray(np.asarray(Wq)[cols, :].T).astype(bf),
            "wkT": np.ascontiguousarray(np.asarray(Wk)[cols, :].T).astype(bf),
            "wvT": np.ascontiguousarray(np.asarray(Wv)[cols, :].T).astype(bf),
            "woT2": np.ascontiguousarray(np.asarray(Wo)[cols, :].T).astype(bf),
            "cosT": cosT, "sinT": sinT, "swapT": swapT, "maskT": maskT,
            "selT": selT,
        })
    return in_maps


def kernel(x, Wq, Wk, Wv, Wo, cos_emb, sin_emb, token_positions, **run_kwargs):
    if "nc" not in _compiled:
        _compiled["nc"] = _build_nc()
    nc = _compiled["nc"]
    in_maps = _prep_inputs(x, Wq, Wk, Wv, Wo, cos_emb, sin_emb, token_positions)
    res = run_bass_kernel_spmd(nc, in_maps, list(range(NCORES)), **run_kwargs)
    out = np.stack([
        np.concatenate([res.results[2 * b]["y"], res.results[2 * b + 1]["y"]],
                       axis=1)
        for b in range(B)
    ]).astype(np.float32)
    if run_kwargs:
        kernel.last_result = res
    return out
